# revision 44
# baseline (speedup 1.0000x reference)
"""BiGCN (nn_BiGCN_52716428591487) Trainium2 kernel.

Math: the model's output is log_softmax(cat(l2_bu[root], l2_td[root]) @ W_lin + b).
Only the layer-2 GCN values AT THE ROOT NODES matter, and GCNConv is linear in
its input features, so the whole network collapses to:

  agg1_d[v]  = sum_{e -> v} coef_d(e) * x[nbr(e)]            (v in S; self loop
                                                              folded in as an
                                                              ordinary edge)
  l1_d[v]    = agg1_d[v] @ W_d1 + b_d1
  cb/ct[v]   = relu([x[root(g(v))], l1_bu/td[v]])
  out2[g]    = sum_{s in S_g} Pr[s, g] * [relu(root), relu(l1_bu), relu(l1_td)][s]
             = [prsum_g * relu(x[root_g]), ...]               (root part exact)
  pb/pt[g]   = relu(out2_{R,bu/td}[g] @ W_2 + b_2)
  out[g]     = log_softmax([pb, pt][g] @ W_lin + b_lin)

where S = {sources of root-incident edges} + {roots} (~1.7k of 50k nodes) and
Pr is the (structure-only) layer-2 aggregation matrix.

Device layout: per core, per chunk (128 S-slots), the union U of rows needed
by BOTH directions is packed into k-tiles of 128. One fp8 tensor `big`
[128, K, F+256] holds, per k-tile, the x rows (cols 0:F) and the two
directions' scatter/coef matrices (cols F:F+256, built host-side from indices
and degrees only). Stage 1 is then plain PSUM-accumulated fp8 DoubleRow
matmuls producing aggT = [F-half, bu-slots | td-slots] directly - no
transposes, no on-device one-hot building. Everything downstream (W1, relu,
Pr, W2, head, log_softmax) runs in bf16/fp32 on device.

Host does index-only preprocessing (degrees, edge selection, row/slot maps,
scatter matrices) plus the x-row gather; the device does every arithmetic op
that touches x.

Sharding: graph-data parallel over 8 cores; the host concatenates per-core
[G_cap, C] outputs.
"""

import numpy as np
import ml_dtypes

P = 128
NCORES = 8
NCHUNK = 4      # chunks per core
CS = 64         # S-slots per chunk; 2 chunks stack into 128 partitions
NPAIR = NCHUNK // 2


# ----------------------------------------------------------------------------
# Host preprocessing: index-only work + gather tables
# ----------------------------------------------------------------------------

def _preprocess(x, edge_index, batch, num_graphs):
    x = np.ascontiguousarray(np.asarray(x), dtype=np.float32)
    ei = np.asarray(edge_index)
    batch = np.asarray(batch).astype(np.int64)
    G = int(np.asarray(num_graphs))
    N, F = x.shape
    src = ei[0].astype(np.int64)
    dst = ei[1].astype(np.int64)

    assert np.all(np.diff(batch) >= 0), "batch must be sorted (contiguous graphs)"
    roots = np.searchsorted(batch, np.arange(G, dtype=np.int64))  # segment_min

    deg_td = 1.0 + np.bincount(dst, minlength=N).astype(np.float64)
    deg_bu = 1.0 + np.bincount(src, minlength=N).astype(np.float64)
    dinv_td = 1.0 / np.sqrt(deg_td)
    dinv_bu = 1.0 / np.sqrt(deg_bu)

    G_cap = max(-(-G // NCORES), 1)

    # S: sources of root-incident edges + roots
    is_root = np.zeros(N, bool)
    is_root[roots] = True
    rmask = is_root[dst]
    r_src, r_dst = src[rmask], dst[rmask]
    r_coef = (dinv_td[r_src] * dinv_td[r_dst]).astype(np.float32)

    s_nodes = np.unique(np.concatenate([r_src, roots]))  # sorted
    s_mask = np.zeros(N, bool)
    s_mask[s_nodes] = True

    # layer-1 edge lists with aggregation target in S (coef excl. self term;
    # self loops are added as explicit (v, v) entries with coef dinv_d[v]^2)
    dirs = {}
    for dn, tgt_all, nbr_all, dinv in (("bu", src, dst, dinv_bu),
                                       ("td", dst, src, dinv_td)):
        sel = s_mask[tgt_all]
        tgt, nbr = tgt_all[sel], nbr_all[sel]
        coef = (dinv[tgt] * dinv[nbr]).astype(np.float32)
        tgt = np.concatenate([tgt, s_nodes])
        nbr = np.concatenate([nbr, s_nodes])
        coef = np.concatenate([coef, (dinv[s_nodes] ** 2).astype(np.float32)])
        dirs[dn] = (tgt, nbr, coef)

    # per-graph union of rows needed (neighbors of S in either dir + S)
    all_nodes = np.concatenate([dirs["bu"][1], dirs["td"][1], s_nodes])
    u_nodes = np.unique(batch[all_nodes] * N + all_nodes)  # (graph, node) keys
    u_graph = u_nodes // N
    u_node = u_nodes % N
    U_g = np.bincount(u_graph, minlength=G)          # union size per graph
    S_g = np.bincount(batch[s_nodes], minlength=G)   # S slots per graph
    assert S_g.max() <= CS

    # graph -> (core, chunk) bin: LPT greedy on union size (which sets the
    # SPMD k-tile count), then local-search moves/swaps to shave the max
    # bin. Constraints: P S-slots per bin, G_cap graphs per core.
    NB = NCORES * NCHUNK
    bin_of = np.empty(G, np.int64)
    bu_ld = np.zeros(NB, np.int64)
    bs_ld = np.zeros(NB, np.int64)
    bc_ld = np.zeros(NCORES, np.int64)
    for g in np.argsort(-U_g, kind="stable"):
        cands = [b for b in range(NB)
                 if bs_ld[b] + S_g[g] <= CS and bc_ld[b // NCHUNK] < G_cap]
        b = min(cands, key=lambda bb: bu_ld[bb])
        bin_of[g] = b
        bu_ld[b] += U_g[g]
        bs_ld[b] += S_g[g]
        bc_ld[b // NCHUNK] += 1

    def _can_host(b, g):
        same_core = b // NCHUNK == bin_of[g] // NCHUNK
        return (bs_ld[b] + S_g[g] <= CS
                and (same_core or bc_ld[b // NCHUNK] < G_cap))

    for _ in range(300):
        mb = int(np.argmax(bu_ld))
        best = None  # (new_pair_max, g, b2)
        for g in np.flatnonzero(bin_of == mb):
            for b2 in range(NB):
                if b2 == mb or not _can_host(b2, g):
                    continue
                nm = max(bu_ld[mb] - U_g[g], bu_ld[b2] + U_g[g])
                if nm < bu_ld[mb] and (best is None or nm < best[0]):
                    best = (nm, g, b2)
        if best is None:
            break
        _, g, b2 = best
        ob = bin_of[g]
        bin_of[g] = b2
        bu_ld[ob] -= U_g[g]; bs_ld[ob] -= S_g[g]; bc_ld[ob // NCHUNK] -= 1
        bu_ld[b2] += U_g[g]; bs_ld[b2] += S_g[g]; bc_ld[b2 // NCHUNK] += 1

    core_of_graph = bin_of // NCHUNK
    chunk_of_graph = bin_of % NCHUNK
    counts = np.bincount(core_of_graph, minlength=NCORES)
    glocal = np.empty(G, np.int64)
    for c in range(NCORES):
        gs = np.flatnonzero(core_of_graph == c)
        glocal[gs] = np.arange(len(gs))

    # uniform k-tile count per chunk (SPMD); odd Kc handled by a trailing
    # non-DoubleRow matmul in stage 1
    Kc = max(int(-(-bu_ld.max() // P)), 2)
    K = NCHUNK * Kc

    # row map (position in the packed k-tiles) and slot map per node
    rowmap = np.full(N, -1, np.int64)
    slotmap = np.full(N, -1, np.int64)
    chunkmap = np.full(N, -1, np.int64)
    roots_of = np.zeros((NCORES, G_cap), np.int64)
    for c in range(NCORES):
        gs = np.flatnonzero(core_of_graph == c)  # ascending graph id
        roots_of[c, :len(gs)] = roots[gs]
        for b in range(NCHUNK):
            rbase = b * Kc * P
            sbase = 0
            for g in gs[chunk_of_graph[gs] == b]:
                un = u_node[u_graph == g]
                rowmap[un] = rbase + np.arange(len(un))
                rbase += len(un)
                sn = s_nodes[batch[s_nodes] == g]
                slotmap[sn] = sbase + np.arange(len(sn))
                sbase += len(sn)
                chunkmap[un] = b
            assert rbase <= (b + 1) * Kc * P and sbase <= CS

    assert F % P == 0
    W8 = F + 2 * CS

    # per-core big fp8 tensor [P, K, F + 256] and Pr/xroot tables
    in_maps = []
    dirP = {"bu": 0, "td": 1}
    for c in range(NCORES):
        big = np.zeros((P, K, W8), np.float32)
        sel_u = core_of_graph[u_graph] == c
        un = u_node[sel_u]
        r = rowmap[un]
        big[r % P, r // P, :F] = x[un]
        for dn, (tgt, nbr, coef) in dirs.items():
            sel = core_of_graph[batch[tgt]] == c
            t, n_, cf = tgt[sel], nbr[sel], coef[sel]
            rr = rowmap[n_]
            np.add.at(big, (rr % P, rr // P, F + dirP[dn] * CS + slotmap[t]), cf)
        m = {"big": big.astype(ml_dtypes.float8_e4m3)}

        # Pr rows stack chunk pairs: partition = (chunk%2)*CS + slot
        Pr = np.zeros((NPAIR, P, G_cap), np.float32)
        sel = core_of_graph[batch[r_dst]] == c
        rs, rd, rc = r_src[sel], r_dst[sel], r_coef[sel]
        np.add.at(Pr, (chunkmap[rs] // 2,
                       (chunkmap[rs] % 2) * CS + slotmap[rs],
                       glocal[batch[rd]]), rc)
        gs = np.flatnonzero(core_of_graph == c)
        np.add.at(Pr, (chunkmap[roots[gs]] // 2,
                       (chunkmap[roots[gs]] % 2) * CS + slotmap[roots[gs]],
                       glocal[gs]),
                  (dinv_td[roots[gs]] ** 2).astype(np.float32))
        m["pr"] = Pr
        m["prsum"] = Pr.sum(axis=(0, 1))                      # [G_cap]
        xr = np.zeros((F, G_cap), np.float32)
        xr[:, :len(gs)] = x[roots[gs]].T
        m["xrootT"] = xr
        in_maps.append(m)

    meta = dict(F=F, K=K, Kc=Kc, G_cap=G_cap, counts=counts, G=G,
                core_of_graph=core_of_graph, glocal=glocal)
    return in_maps, meta


def _const_layout(F, H, C, G_cap):
    """Column layout of the fused per-core bf16 constant matrix [P, W]."""
    nF = F // P
    off = 0
    L = {}

    def add(name, w):
        nonlocal off
        L[name] = (off, w)
        off += w

    for d in ("bu", "td"):
        for h in range(nF):
            add(f"w1{d}{h}", H)
    for d in ("bu", "td"):
        for h in range(nF):
            add(f"w2rf{d}{h}", H)
        add(f"w2l1{d}", H)
    for d in ("bu", "td"):
        add(f"wl{d}", C)
    # bias rows live at matmul-legal base partitions 0/32/64:
    # biasA rows: 0=b1bu 32=b1td 64=b2bu; biasB rows: 0=b2td 32=bl.
    # ones is replicated at rows 0/32/64 so either operand of a rank-1
    # bias matmul can sit at the same base partition as its partner.
    add("biasA", H)
    add("biasB", max(C, H))
    add("ones", P)
    for t in range(NPAIR):
        add(f"pr{t}", G_cap)
    add("prsum", G_cap)
    for h in range(nF):
        add(f"xrootT{h}", G_cap)
    return L, off


def _pack_consts(in_maps, inputs, meta):
    H = int(np.asarray(inputs["W_td1"]).shape[1])
    C = int(np.asarray(inputs["W_lin"]).shape[1])
    F, G_cap = meta["F"], meta["G_cap"]
    nF = F // P
    assert H == P
    L, W = _const_layout(F, H, C, G_cap)
    g = lambda k: np.asarray(inputs[k], dtype=np.float32)

    base = np.zeros((P, W), np.float32)

    def put(name, block, row0=0):
        o, w = L[name]
        block = np.asarray(block, np.float32)
        base[row0:row0 + block.shape[0], o:o + block.shape[1]] = block

    for d, w1n, w2n in (("bu", "W_bu1", "W_bu2"), ("td", "W_td1", "W_td2")):
        for h in range(nF):
            put(f"w1{d}{h}", g(w1n)[h * P:(h + 1) * P, :])
            put(f"w2rf{d}{h}", g(w2n)[h * P:(h + 1) * P, :])
        put(f"w2l1{d}", g(w2n)[F:F + H, :])
    put("wlbu", g("W_lin")[0:H, :])
    put("wltd", g("W_lin")[H:2 * H, :])
    put("biasA", g("b_bu1").reshape(1, H), row0=0)
    put("biasA", g("b_td1").reshape(1, H), row0=32)
    put("biasA", g("b_bu2").reshape(1, H), row0=64)
    put("biasB", g("b_td2").reshape(1, H), row0=0)
    put("biasB", g("b_lin").reshape(1, C), row0=32)
    for r in (0, 32, 64):
        put("ones", np.ones((1, P), np.float32), row0=r)

    for m in in_maps:
        cst = base.copy()
        Pr = m.pop("pr")
        for t in range(NPAIR):
            o, w = L[f"pr{t}"]
            cst[:, o:o + w] = Pr[t]
        o, w = L["prsum"]
        cst[:, o:o + w] = m.pop("prsum")[None, :]
        xr = m.pop("xrootT")
        for h in range(nF):
            o, w = L[f"xrootT{h}"]
            cst[:, o:o + w] = xr[h * P:(h + 1) * P, :]
        m["cst"] = cst.astype(ml_dtypes.bfloat16)
    meta["H"], meta["C"] = H, C
    return H, C


# ----------------------------------------------------------------------------
# Device program
# ----------------------------------------------------------------------------

def _build_program(F, H, C, G_cap, Kc, repeat=1):
    from contextlib import ExitStack

    import concourse.bacc as bacc
    import concourse.bass as bass  # noqa: F401
    import concourse.mybir as mybir
    import concourse.tile as tile

    dt = mybir.dt.float32
    dtb = mybir.dt.bfloat16
    dt8 = mybir.dt.float8e4
    K = NCHUNK * Kc
    nF = F // P
    W8 = F + 2 * CS
    assert F % P == 0 and H == P and nF == 2
    L, W = _const_layout(F, H, C, G_cap)

    nc = bacc.Bacc("TRN2", target_bir_lowering=False, debug=False,
                   num_devices=NCORES)

    big_d = nc.dram_tensor("big", [P, K, W8], dt8, kind="ExternalInput").ap()
    cst_d = nc.dram_tensor("cst", [P, W], dtb, kind="ExternalInput").ap()
    out = nc.dram_tensor("out", [G_cap, C], dt, kind="ExternalOutput").ap()

    mx, sub = mybir.AluOpType.max, mybir.AluOpType.subtract
    mul, add = mybir.AluOpType.mult, mybir.AluOpType.add
    Exp = mybir.ActivationFunctionType.Exp
    DR = mybir.MatmulPerfMode.DoubleRow

    # ln(s) ~= P(u), u = s/4 - 1, fit over s in [1.8, 9]. The logits are
    # O(0.1) so s = sum_j exp(lg_j) stays near C=4; the fit residual is
    # ~1e-5, far under the tolerance. Evaluating ln on the DVE keeps the
    # ACT engine on the single exp_and_others table (copy+exp), avoiding a
    # 2x1.3us act-table reload every rep that exp+ln would cost.
    ss = np.linspace(2.2, 7.0, 4001)
    uu = ss / 4.0 - 1.0
    lncoef = np.polyfit(uu, np.log(ss), 7)
    assert np.abs(np.polyval(lncoef, uu) - np.log(ss)).max() < 5e-5

    with ExitStack() as ctx:
        tc = ctx.enter_context(tile.TileContext(nc))
        bufs2 = 1 if repeat == 1 else 3
        const = ctx.enter_context(tc.tile_pool(name="const", bufs=bufs2))
        bpool = ctx.enter_context(tc.tile_pool(name="bp", bufs=bufs2))
        apool = ctx.enter_context(tc.tile_pool(name="ap", bufs=8))
        cpool = ctx.enter_context(tc.tile_pool(name="cp", bufs=4))
        spool = ctx.enter_context(tc.tile_pool(name="sp", bufs=8))
        ps = ctx.enter_context(tc.tile_pool(name="ps", bufs=4, space="PSUM"))
        ps2 = ctx.enter_context(tc.tile_pool(name="ps2", bufs=4, space="PSUM"))

        for _rep in range(repeat):
            # split the big transfer across BOTH hardware DGE queues (SP and
            # ACT): each queue drives its own set of DMA engines, so the two
            # halves stream concurrently
            cst = const.tile([P, W], dtb, name="cst", tag="cst")
            nc.scalar.dma_start(cst[:], cst_d[:])
            big = bpool.tile([P, K, W8], dt8, name="big", tag="big")
            nc.sync.dma_start(big[:, 0:K // 2, :], big_d[:, 0:K // 2, :])
            nc.scalar.dma_start(big[:, K // 2:, :], big_d[:, K // 2:, :])

            def C_(name, rows=None):
                o, w = L[name]
                if rows is None:
                    return cst[:, o:o + w]
                return cst[rows, o:o + w]

            def B_(block, row, width):
                o, _ = L[block]
                return cst[row:row + 1, o:o + width]

            # bias rows (base partitions 0/32/64 as matmul requires); the
            # ones row is replicated so it can match its partner's row
            b1r = [B_("biasA", 0, H), B_("biasA", 32, H)]    # b1bu, b1td
            b1ones = [B_("ones", 0, P), B_("ones", 32, P)]
            b2r = [B_("biasA", 64, H), B_("biasB", 0, H)]    # b2bu, b2td
            b2ones = [B_("ones", 64, G_cap), B_("ones", 0, G_cap)]
            blr = B_("biasB", 32, C)
            blones = B_("ones", 32, G_cap)

            # stage 1: per chunk pair tile [P, 512]: cols (c%2)*256 + h*128
            # hold aggT[f-half h, bu CS | td CS] for chunk c. fp8 DoubleRow
            # matmuls over k-tile pairs; one accumulation group per tile
            # (first start marks the whole 2KB zero region; later start=False
            # matmuls read-as-zero on first touch per byte).
            agg_ps = [ps.tile([P, 4 * P], dt, tag="ps", name="aggps")
                      for _ in range(NPAIR)]
            for c in range(NCHUNK):
                for jp in range(-(-Kc // 2)):
                    j = c * Kc + 2 * jp
                    pair = 2 * jp + 1 < Kc  # odd Kc: last tile is unpaired
                    first = c % 2 == 0 and jp == 0
                    stop = (c % 2 == 1 and jp == -(-Kc // 2) - 1)
                    o_c = (c % 2) * 2 * P
                    for h in range(nF):
                        if pair:
                            nc.tensor.matmul(
                                out=agg_ps[c // 2][:, o_c + h * P:
                                                   o_c + (h + 1) * P],
                                lhsT=big[:, j:j + 2, h * P:(h + 1) * P],
                                rhs=big[:, j:j + 2, F:F + 2 * CS],
                                start=(first and h == 0),
                                stop=(stop and h == nF - 1),
                                perf_mode=DR, skip_group_check=True)
                        else:
                            nc.tensor.matmul(
                                out=agg_ps[c // 2][:, o_c + h * P:
                                                   o_c + (h + 1) * P],
                                lhsT=big[:, j, h * P:(h + 1) * P],
                                rhs=big[:, j, F:F + 2 * CS],
                                start=(first and h == 0),
                                stop=(stop and h == nF - 1),
                                skip_group_check=True)
            # PSUM -> SBUF (bf16), one full-bank copy per pair, spread over
            # ACT and DVE engines (GPSIMD cannot read PSUM)
            aggT = []
            for t_ in range(NPAIR):
                t = apool.tile([P, 4 * P], dtb, tag="aggT", name="aggT")
                if t_ == 0:
                    nc.scalar.copy(t[:], agg_ps[t_][:])
                else:
                    nc.vector.tensor_copy(out=t[:], in_=agg_ps[t_][:])
                aggT.append(t)

            def aggT_sl(c, h, di):
                o = (c % 2) * 2 * P + h * P + di * CS
                return aggT[c // 2][:, o:o + CS]

            # stage 2: l1[c][d] = aggT_d^T @ W1_d + b1_d, relu -> cbt.
            # Chunk pairs stack on the partition axis: chunk c writes
            # partitions (c%2)*CS..+CS of tile c//2, cols d*H..+H. One
            # accumulation group per (tile, partition half) - zero regions
            # are per partition row, so the halves don't clobber each other.
            cbt = [cpool.tile([P, 2 * H], dtb, tag="cbt", name="cbt")
                   for _ in range(NPAIR)]
            l1 = [ps2.tile([P, 2 * H], dt, tag="ps2", name="l1ps")
                  for _ in range(NPAIR)]
            for c in range(NCHUNK):
                pb = (c % 2) * CS
                for di, d in enumerate(("bu", "td")):
                    sl = l1[c // 2][pb:pb + CS, di * H:(di + 1) * H]
                    for h in range(nF):
                        nc.tensor.matmul(
                            out=sl, lhsT=aggT_sl(c, h, di),
                            rhs=C_(f"w1{d}{h}"),
                            start=(di == 0 and h == 0), stop=False,
                            skip_group_check=True)
                    nc.tensor.matmul(out=sl, lhsT=b1ones[di][:, :CS],
                                     rhs=b1r[di], start=False, stop=(di == 1),
                                     skip_group_check=True)
            for t_ in range(NPAIR):
                nc.vector.tensor_scalar(
                    out=cbt[t_][:], in0=l1[t_][:],
                    scalar1=0.0, scalar2=None, op0=mx)

            # stage 4: o2T[d] [P(l1_d cols), G_cap] = cbt_d^T @ Pr; both
            # directions pack into one PSUM bank (single merged group, 16-col
            # stride) and leave PSUM in one copy.
            GS = 16
            o2 = ps2.tile([P, 2 * GS], dt, tag="ps2", name="o2ps")
            for di in range(2):
                for t_ in range(NPAIR):
                    nc.tensor.matmul(out=o2[:, di * GS:di * GS + G_cap],
                                     lhsT=cbt[t_][:, di * H:(di + 1) * H],
                                     rhs=C_(f"pr{t_}"),
                                     start=(di == 0 and t_ == 0),
                                     stop=(di == 1 and t_ == NPAIR - 1),
                                     skip_group_check=True)
            o2T = spool.tile([P, 2 * GS], dtb, tag="o2T", name="o2T")
            nc.vector.tensor_copy(out=o2T[:], in_=o2[:])

            # root-feature part: rfs[h] = relu(xrootT_h) * prsum (per column)
            rfs = []
            for h in range(nF):
                t = spool.tile([P, G_cap], dtb, tag=f"rfs{h}", name="rfs")
                nc.vector.scalar_tensor_tensor(
                    out=t[:], in0=C_(f"xrootT{h}"), scalar=0.0,
                    in1=C_("prsum"), op0=mx, op1=mul)
                rfs.append(t)

            # stage 5: tot[d] [H, G_cap] = relu(W2_d^T @ [rfs | o2T_d] + b2_d)
            # Both directions pack into one PSUM bank + one relu.
            tp = ps2.tile([P, 2 * GS], dt, tag="ps2", name="totps")
            for di, d in enumerate(("bu", "td")):
                sl = tp[:, di * GS:di * GS + G_cap]
                for h in range(nF):
                    nc.tensor.matmul(out=sl, lhsT=C_(f"w2rf{d}{h}"),
                                     rhs=rfs[h][:],
                                     start=(di == 0 and h == 0), stop=False,
                                     skip_group_check=True)
                nc.tensor.matmul(out=sl, lhsT=C_(f"w2l1{d}"),
                                 rhs=o2T[:, di * GS:di * GS + G_cap],
                                 start=False, stop=False,
                                 skip_group_check=True)
                nc.tensor.matmul(out=sl, lhsT=b2r[di],
                                 rhs=b2ones[di], start=False, stop=(di == 1),
                                 skip_group_check=True)
            tot = spool.tile([P, 2 * GS], dtb, tag="tot", name="tot")
            nc.vector.tensor_scalar(out=tot[:], in0=tp[:], scalar1=0.0,
                                    scalar2=None, op0=mx)

            # stage 6: logits [G_cap, C] = tot^T @ W_lin + b_lin
            lg = ps2.tile([G_cap, C], dt, tag="ps2", name="lgps")
            for di, d in enumerate(("bu", "td")):
                nc.tensor.matmul(out=lg[:], lhsT=tot[:, di * GS:di * GS + G_cap],
                                 rhs=C_(f"wl{d}"), start=(di == 0), stop=False)
            nc.tensor.matmul(out=lg[:], lhsT=blones,
                             rhs=blr, start=False, stop=True)

            # log_softmax rows: exp+accum on ACT, then ln(s) via the DVE
            # Horner polynomial above (logits are O(1): no max-subtraction).
            ez = spool.tile([G_cap, C], dt, tag="ez", name="ez")
            se = spool.tile([G_cap, 1], dt, tag="se", name="se")
            nc.scalar.activation(ez[:], lg[:], Exp, accum_out=se[:])
            uu_t = spool.tile([G_cap, 1], dt, tag="uu", name="uu")
            nc.vector.tensor_scalar(out=uu_t[:], in0=se[:], scalar1=0.25,
                                    scalar2=-1.0, op0=mul, op1=add)
            lse = spool.tile([G_cap, 1], dt, tag="lse", name="lse")
            nc.vector.tensor_scalar(out=lse[:], in0=uu_t[:],
                                    scalar1=float(lncoef[0]),
                                    scalar2=float(lncoef[1]), op0=mul, op1=add)
            for ck in lncoef[2:]:
                nc.vector.tensor_scalar(out=lse[:], in0=lse[:],
                                        scalar1=uu_t[:], scalar2=float(ck),
                                        op0=mul, op1=add)
            res = spool.tile([G_cap, C], dt, tag="res", name="res")
            nc.vector.tensor_scalar(out=res[:], in0=lg[:], scalar1=lse[:],
                                    scalar2=None, op0=sub)
            nc.sync.dma_start(out[:], res[:])

    nc.compile()
    return nc


_PROG_CACHE = {}


def _prepare_maps(inputs):
    in_maps, meta = _preprocess(inputs["x"], inputs["edge_index"],
                                inputs["batch"], inputs["num_graphs"])
    _pack_consts(in_maps, inputs, meta)
    meta["key"] = (meta["F"], meta["H"], meta["C"], meta["G_cap"], meta["Kc"])
    return in_maps, meta


def _prepare(inputs):
    in_maps, meta = _prepare_maps(inputs)
    key = meta["key"]
    if key not in _PROG_CACHE:
        _PROG_CACHE[key] = _build_program(*key)
    return _PROG_CACHE[key], in_maps, meta


def kernel(**inputs):
    from concourse.bass_utils import run_bass_kernel_spmd

    nc, in_maps, meta = _prepare(inputs)
    res = run_bass_kernel_spmd(nc, in_maps, list(range(NCORES)))
    G = meta["G"]
    cog, gl = meta["core_of_graph"], meta["glocal"]
    out = np.empty((G, meta["C"]), np.float32)
    for g in range(G):
        out[g] = res.results[cog[g]]["out"][gl[g]]
    return out


# revision 45
# speedup vs baseline: 1.3732x; 1.3732x over previous
"""BiGCN (nn_BiGCN_52716428591487) Trainium2 kernel.

Math: the model's output is log_softmax(cat(l2_bu[root], l2_td[root]) @ W_lin + b).
Only the layer-2 GCN values AT THE ROOT NODES matter, and GCNConv is linear in
its input features, so the whole network collapses to:

  agg1_d[v]  = sum_{e -> v} coef_d(e) * x[nbr(e)]            (v in S; self loop
                                                              folded in as an
                                                              ordinary edge)
  l1_d[v]    = agg1_d[v] @ W_d1 + b_d1
  cb/ct[v]   = relu([x[root(g(v))], l1_bu/td[v]])
  out2[g]    = sum_{s in S_g} Pr[s, g] * [relu(root), relu(l1_bu), relu(l1_td)][s]
             = [prsum_g * relu(x[root_g]), ...]               (root part exact)
  pb/pt[g]   = relu(out2_{R,bu/td}[g] @ W_2 + b_2)
  out[g]     = log_softmax([pb, pt][g] @ W_lin + b_lin)

where S = {sources of root-incident edges} + {roots} (~1.7k of 50k nodes) and
Pr is the (structure-only) layer-2 aggregation matrix.

Device layout: per core, per chunk (128 S-slots), the union U of rows needed
by BOTH directions is packed into k-tiles of 128. One fp8 tensor `big`
[128, K, F+256] holds, per k-tile, the x rows (cols 0:F) and the two
directions' scatter/coef matrices (cols F:F+256, built host-side from indices
and degrees only). Stage 1 is then plain PSUM-accumulated fp8 DoubleRow
matmuls producing aggT = [F-half, bu-slots | td-slots] directly - no
transposes, no on-device one-hot building. Everything downstream (W1, relu,
Pr, W2, head, log_softmax) runs in bf16/fp32 on device.

Host does index-only preprocessing (degrees, edge selection, row/slot maps,
scatter matrices) plus the x-row gather; the device does every arithmetic op
that touches x.

Sharding: graph-data parallel over 8 cores; the host concatenates per-core
[G_cap, C] outputs.
"""

import numpy as np
import ml_dtypes

P = 128
NCORES = 8
NCHUNK = 2


# ----------------------------------------------------------------------------
# Host preprocessing: index-only work + gather tables
# ----------------------------------------------------------------------------

def _preprocess(x, edge_index, batch, num_graphs):
    x = np.ascontiguousarray(np.asarray(x), dtype=np.float32)
    ei = np.asarray(edge_index)
    batch = np.asarray(batch).astype(np.int64)
    G = int(np.asarray(num_graphs))
    N, F = x.shape
    src = ei[0].astype(np.int64)
    dst = ei[1].astype(np.int64)

    assert np.all(np.diff(batch) >= 0), "batch must be sorted (contiguous graphs)"
    roots = np.searchsorted(batch, np.arange(G, dtype=np.int64))  # segment_min

    deg_td = 1.0 + np.bincount(dst, minlength=N).astype(np.float64)
    deg_bu = 1.0 + np.bincount(src, minlength=N).astype(np.float64)
    dinv_td = 1.0 / np.sqrt(deg_td)
    dinv_bu = 1.0 / np.sqrt(deg_bu)

    G_cap = max(-(-G // NCORES), 1)

    # S: sources of root-incident edges + roots
    is_root = np.zeros(N, bool)
    is_root[roots] = True
    rmask = is_root[dst]
    r_src, r_dst = src[rmask], dst[rmask]
    r_coef = (dinv_td[r_src] * dinv_td[r_dst]).astype(np.float32)

    s_nodes = np.unique(np.concatenate([r_src, roots]))  # sorted
    s_mask = np.zeros(N, bool)
    s_mask[s_nodes] = True

    # layer-1 edge lists with aggregation target in S (coef excl. self term;
    # self loops are added as explicit (v, v) entries with coef dinv_d[v]^2)
    dirs = {}
    for dn, tgt_all, nbr_all, dinv in (("bu", src, dst, dinv_bu),
                                       ("td", dst, src, dinv_td)):
        sel = s_mask[tgt_all]
        tgt, nbr = tgt_all[sel], nbr_all[sel]
        coef = (dinv[tgt] * dinv[nbr]).astype(np.float32)
        tgt = np.concatenate([tgt, s_nodes])
        nbr = np.concatenate([nbr, s_nodes])
        coef = np.concatenate([coef, (dinv[s_nodes] ** 2).astype(np.float32)])
        dirs[dn] = (tgt, nbr, coef)

    # per-graph union of rows needed (neighbors of S in either dir + S)
    all_nodes = np.concatenate([dirs["bu"][1], dirs["td"][1], s_nodes])
    u_nodes = np.unique(batch[all_nodes] * N + all_nodes)  # (graph, node) keys
    u_graph = u_nodes // N
    u_node = u_nodes % N
    U_g = np.bincount(u_graph, minlength=G)          # union size per graph
    S_g = np.bincount(batch[s_nodes], minlength=G)   # S slots per graph
    assert S_g.max() <= P

    # graph -> (core, chunk) bin: LPT greedy on union size (which sets the
    # SPMD k-tile count), then local-search moves/swaps to shave the max
    # bin. Constraints: P S-slots per bin, G_cap graphs per core.
    NB = NCORES * NCHUNK
    bin_of = np.empty(G, np.int64)
    bu_ld = np.zeros(NB, np.int64)
    bs_ld = np.zeros(NB, np.int64)
    bc_ld = np.zeros(NCORES, np.int64)
    for g in np.argsort(-U_g, kind="stable"):
        cands = [b for b in range(NB)
                 if bs_ld[b] + S_g[g] <= P and bc_ld[b // NCHUNK] < G_cap]
        b = min(cands, key=lambda bb: bu_ld[bb])
        bin_of[g] = b
        bu_ld[b] += U_g[g]
        bs_ld[b] += S_g[g]
        bc_ld[b // NCHUNK] += 1

    def _can_host(b, g):
        same_core = b // NCHUNK == bin_of[g] // NCHUNK
        return (bs_ld[b] + S_g[g] <= P
                and (same_core or bc_ld[b // NCHUNK] < G_cap))

    for _ in range(300):
        mb = int(np.argmax(bu_ld))
        best = None  # (new_pair_max, g, b2)
        for g in np.flatnonzero(bin_of == mb):
            for b2 in range(NB):
                if b2 == mb or not _can_host(b2, g):
                    continue
                nm = max(bu_ld[mb] - U_g[g], bu_ld[b2] + U_g[g])
                if nm < bu_ld[mb] and (best is None or nm < best[0]):
                    best = (nm, g, b2)
        if best is None:
            break
        _, g, b2 = best
        ob = bin_of[g]
        bin_of[g] = b2
        bu_ld[ob] -= U_g[g]; bs_ld[ob] -= S_g[g]; bc_ld[ob // NCHUNK] -= 1
        bu_ld[b2] += U_g[g]; bs_ld[b2] += S_g[g]; bc_ld[b2 // NCHUNK] += 1

    core_of_graph = bin_of // NCHUNK
    chunk_of_graph = bin_of % NCHUNK
    counts = np.bincount(core_of_graph, minlength=NCORES)
    glocal = np.empty(G, np.int64)
    for c in range(NCORES):
        gs = np.flatnonzero(core_of_graph == c)
        glocal[gs] = np.arange(len(gs))

    # uniform k-tile count per chunk (SPMD); odd Kc handled by a trailing
    # non-DoubleRow matmul in stage 1
    Kc = max(int(-(-bu_ld.max() // P)), 2)
    K = NCHUNK * Kc

    # row map (position in the packed k-tiles) and slot map per node
    rowmap = np.full(N, -1, np.int64)
    slotmap = np.full(N, -1, np.int64)
    chunkmap = np.full(N, -1, np.int64)
    roots_of = np.zeros((NCORES, G_cap), np.int64)
    for c in range(NCORES):
        gs = np.flatnonzero(core_of_graph == c)  # ascending graph id
        roots_of[c, :len(gs)] = roots[gs]
        for b in range(NCHUNK):
            rbase = b * Kc * P
            sbase = 0
            for g in gs[chunk_of_graph[gs] == b]:
                un = u_node[u_graph == g]
                rowmap[un] = rbase + np.arange(len(un))
                rbase += len(un)
                sn = s_nodes[batch[s_nodes] == g]
                slotmap[sn] = sbase + np.arange(len(sn))
                sbase += len(sn)
                chunkmap[un] = b
            assert rbase <= (b + 1) * Kc * P and sbase <= P

    F_half = F // P
    assert F % P == 0
    W8 = F + 2 * P

    # per-core big fp8 tensor [P, K, F + 256] and Pr/xroot tables
    in_maps = []
    dirP = {"bu": 0, "td": 1}
    for c in range(NCORES):
        big = np.zeros((P, K, W8), np.float32)
        sel_u = core_of_graph[u_graph] == c
        un = u_node[sel_u]
        r = rowmap[un]
        big[r % P, r // P, :F] = x[un]
        for dn, (tgt, nbr, coef) in dirs.items():
            sel = core_of_graph[batch[tgt]] == c
            t, n_, cf = tgt[sel], nbr[sel], coef[sel]
            rr = rowmap[n_]
            np.add.at(big, (rr % P, rr // P, F + dirP[dn] * P + slotmap[t]), cf)
        m = {"big": big.astype(ml_dtypes.float8_e4m3)}

        Pr = np.zeros((NCHUNK, P, G_cap), np.float32)
        sel = core_of_graph[batch[r_dst]] == c
        rs, rd, rc = r_src[sel], r_dst[sel], r_coef[sel]
        np.add.at(Pr, (chunkmap[rs], slotmap[rs], glocal[batch[rd]]), rc)
        gs = np.flatnonzero(core_of_graph == c)
        np.add.at(Pr, (chunkmap[roots[gs]], slotmap[roots[gs]], glocal[gs]),
                  (dinv_td[roots[gs]] ** 2).astype(np.float32))
        m["pr"] = Pr
        m["prsum"] = Pr.sum(axis=(0, 1))                      # [G_cap]
        xr = np.zeros((F, G_cap), np.float32)
        xr[:, :len(gs)] = x[roots[gs]].T
        m["xrootT"] = xr
        in_maps.append(m)

    meta = dict(F=F, K=K, Kc=Kc, G_cap=G_cap, counts=counts, G=G,
                core_of_graph=core_of_graph, glocal=glocal)
    return in_maps, meta


def _const_layout(F, H, C, G_cap):
    """Column layout of the fused per-core bf16 constant matrix [P, W]."""
    nF = F // P
    off = 0
    L = {}

    def add(name, w):
        nonlocal off
        L[name] = (off, w)
        off += w

    for d in ("bu", "td"):
        for h in range(nF):
            add(f"w1{d}{h}", H)
    for d in ("bu", "td"):
        for h in range(nF):
            add(f"w2rf{d}{h}", H)
        add(f"w2l1{d}", H)
    for d in ("bu", "td"):
        add(f"wl{d}", C)
    # bias rows live at matmul-legal base partitions 0/32/64:
    # biasA rows: 0=b1bu 32=b1td 64=b2bu; biasB rows: 0=b2td 32=bl.
    # ones is replicated at rows 0/32/64 so either operand of a rank-1
    # bias matmul can sit at the same base partition as its partner.
    add("biasA", H)
    add("biasB", max(C, H))
    add("ones", P)
    for c in range(NCHUNK):
        add(f"pr{c}", G_cap)
    add("prsum", G_cap)
    for h in range(nF):
        add(f"xrootT{h}", G_cap)
    return L, off


def _pack_consts(in_maps, inputs, meta):
    H = int(np.asarray(inputs["W_td1"]).shape[1])
    C = int(np.asarray(inputs["W_lin"]).shape[1])
    F, G_cap = meta["F"], meta["G_cap"]
    nF = F // P
    assert H == P
    L, W = _const_layout(F, H, C, G_cap)
    g = lambda k: np.asarray(inputs[k], dtype=np.float32)

    base = np.zeros((P, W), np.float32)

    def put(name, block, row0=0):
        o, w = L[name]
        block = np.asarray(block, np.float32)
        base[row0:row0 + block.shape[0], o:o + block.shape[1]] = block

    for d, w1n, w2n in (("bu", "W_bu1", "W_bu2"), ("td", "W_td1", "W_td2")):
        for h in range(nF):
            put(f"w1{d}{h}", g(w1n)[h * P:(h + 1) * P, :])
            put(f"w2rf{d}{h}", g(w2n)[h * P:(h + 1) * P, :])
        put(f"w2l1{d}", g(w2n)[F:F + H, :])
    put("wlbu", g("W_lin")[0:H, :])
    put("wltd", g("W_lin")[H:2 * H, :])
    put("biasA", g("b_bu1").reshape(1, H), row0=0)
    put("biasA", g("b_td1").reshape(1, H), row0=32)
    put("biasA", g("b_bu2").reshape(1, H), row0=64)
    put("biasB", g("b_td2").reshape(1, H), row0=0)
    put("biasB", g("b_lin").reshape(1, C), row0=32)
    for r in (0, 32, 64):
        put("ones", np.ones((1, P), np.float32), row0=r)

    for m in in_maps:
        cst = base.copy()
        Pr = m.pop("pr")
        for c in range(NCHUNK):
            o, w = L[f"pr{c}"]
            cst[:, o:o + w] = Pr[c]
        o, w = L["prsum"]
        cst[:, o:o + w] = m.pop("prsum")[None, :]
        xr = m.pop("xrootT")
        for h in range(nF):
            o, w = L[f"xrootT{h}"]
            cst[:, o:o + w] = xr[h * P:(h + 1) * P, :]
        m["cst"] = cst.astype(ml_dtypes.bfloat16)
    meta["H"], meta["C"] = H, C
    return H, C


# ----------------------------------------------------------------------------
# Device program
# ----------------------------------------------------------------------------

def _build_program(F, H, C, G_cap, Kc, repeat=1):
    from contextlib import ExitStack

    import concourse.bacc as bacc
    import concourse.bass as bass  # noqa: F401
    import concourse.mybir as mybir
    import concourse.tile as tile

    dt = mybir.dt.float32
    dtb = mybir.dt.bfloat16
    dt8 = mybir.dt.float8e4
    K = NCHUNK * Kc
    nF = F // P
    W8 = F + 2 * P
    assert F % P == 0 and H == P and nF == 2 and Kc % 2 == 0
    L, W = _const_layout(F, H, C, G_cap)

    nc = bacc.Bacc("TRN2", target_bir_lowering=False, debug=False,
                   num_devices=NCORES)

    big_d = nc.dram_tensor("big", [P, K, W8], dt8, kind="ExternalInput").ap()
    cst_d = nc.dram_tensor("cst", [P, W], dtb, kind="ExternalInput").ap()
    out = nc.dram_tensor("out", [G_cap, C], dt, kind="ExternalOutput").ap()

    mx, sub = mybir.AluOpType.max, mybir.AluOpType.subtract
    mul, add = mybir.AluOpType.mult, mybir.AluOpType.add
    Exp = mybir.ActivationFunctionType.Exp
    DR = mybir.MatmulPerfMode.DoubleRow

    # ln(s) ~= P(u), u = s/4 - 1, fit over s in [1.8, 9]. The logits are
    # O(0.1) so s = sum_j exp(lg_j) stays near C=4; the fit residual is
    # ~1e-5, far under the tolerance. Evaluating ln on the DVE keeps the
    # ACT engine on the single exp_and_others table (copy+exp), avoiding a
    # 2x1.3us act-table reload every rep that exp+ln would cost.
    ss = np.linspace(2.2, 7.0, 4001)
    uu = ss / 4.0 - 1.0
    lncoef = np.polyfit(uu, np.log(ss), 7)
    assert np.abs(np.polyval(lncoef, uu) - np.log(ss)).max() < 5e-5

    with ExitStack() as ctx:
        tc = ctx.enter_context(tile.TileContext(nc))
        bufs2 = 1 if repeat == 1 else 3
        const = ctx.enter_context(tc.tile_pool(name="const", bufs=bufs2))
        bpool = ctx.enter_context(tc.tile_pool(name="bp", bufs=bufs2))
        apool = ctx.enter_context(tc.tile_pool(name="ap", bufs=8))
        cpool = ctx.enter_context(tc.tile_pool(name="cp", bufs=4))
        spool = ctx.enter_context(tc.tile_pool(name="sp", bufs=8))
        ps = ctx.enter_context(tc.tile_pool(name="ps", bufs=4, space="PSUM"))
        ps2 = ctx.enter_context(tc.tile_pool(name="ps2", bufs=4, space="PSUM"))

        for _rep in range(repeat):
            # split the big transfer across BOTH hardware DGE queues (SP and
            # ACT): each queue drives its own set of DMA engines, so the two
            # halves stream concurrently
            cst = const.tile([P, W], dtb, name="cst", tag="cst")
            nc.scalar.dma_start(cst[:], cst_d[:])
            big = bpool.tile([P, K, W8], dt8, name="big", tag="big")
            nc.sync.dma_start(big[:, 0:Kc, :], big_d[:, 0:Kc, :])
            nc.scalar.dma_start(big[:, Kc:, :], big_d[:, Kc:, :])

            def C_(name, rows=None):
                o, w = L[name]
                if rows is None:
                    return cst[:, o:o + w]
                return cst[rows, o:o + w]

            def B_(block, row, width):
                o, _ = L[block]
                return cst[row:row + 1, o:o + width]

            # bias rows (base partitions 0/32/64 as matmul requires); the
            # ones row is replicated so it can match its partner's row
            b1r = [B_("biasA", 0, H), B_("biasA", 32, H)]    # b1bu, b1td
            b1ones = [B_("ones", 0, P), B_("ones", 32, P)]
            b2r = [B_("biasA", 64, H), B_("biasB", 0, H)]    # b2bu, b2td
            b2ones = [B_("ones", 64, G_cap), B_("ones", 0, G_cap)]
            blr = B_("biasB", 32, C)
            blones = B_("ones", 32, G_cap)

            # stage 1: aggT[c] [P(F-half), h*256 + (bu slots | td slots)],
            # fp8 DoubleRow matmuls over k-tile pairs of chunk c. Both
            # F-halves accumulate into ONE full-bank PSUM tile as a single
            # group: the first start marks the whole 2KB zero region, later
            # matmuls (start=False) read-as-zero on first touch per byte.
            agg_ps = [ps.tile([P, 4 * P], dt, tag="ps", name="aggps")
                      for _ in range(NCHUNK)]
            for c in range(NCHUNK):
                for jp in range(-(-Kc // 2)):
                    j = c * Kc + 2 * jp
                    pair = 2 * jp + 1 < Kc  # odd Kc: last tile is unpaired
                    last = jp == -(-Kc // 2) - 1
                    for h in range(nF):
                        if pair:
                            nc.tensor.matmul(
                                out=agg_ps[c][:, h * 2 * P:(h + 1) * 2 * P],
                                lhsT=big[:, j:j + 2, h * P:(h + 1) * P],
                                rhs=big[:, j:j + 2, F:F + 2 * P],
                                start=(jp == 0 and h == 0),
                                stop=(last and h == nF - 1),
                                perf_mode=DR, skip_group_check=True)
                        else:
                            nc.tensor.matmul(
                                out=agg_ps[c][:, h * 2 * P:(h + 1) * 2 * P],
                                lhsT=big[:, j, h * P:(h + 1) * P],
                                rhs=big[:, j, F:F + 2 * P],
                                start=(jp == 0 and h == 0),
                                stop=(last and h == nF - 1),
                                skip_group_check=True)
            # PSUM -> SBUF (bf16), one full-bank copy per chunk, spread over
            # ACT and DVE engines (GPSIMD cannot read PSUM)
            aggT = []
            for c in range(NCHUNK):
                t = apool.tile([P, 4 * P], dtb, tag="aggT", name="aggT")
                if c == 0:
                    nc.scalar.copy(t[:], agg_ps[c][:])
                else:
                    nc.vector.tensor_copy(out=t[:], in_=agg_ps[c][:])
                aggT.append(t)

            def aggT_sl(c, h, di):
                return aggT[c][:, h * 2 * P + di * P:h * 2 * P + (di + 1) * P]

            # stage 2: l1[c][d] = aggT_d^T @ W1_d + b1_d, relu -> cbt.
            # All four l1 psums pack into one bank as a single merged group;
            # per chunk one 256-wide relu produces cbt[c] = [bu | td] cols.
            cbt = [cpool.tile([P, 2 * H], dtb, tag="cbt", name="cbt")
                   for _ in range(NCHUNK)]
            l1 = ps2.tile([P, 4 * H], dt, tag="ps2", name="l1ps")
            for c in range(NCHUNK):
                for di, d in enumerate(("bu", "td")):
                    sl = l1[:, (c * 2 + di) * H:(c * 2 + di + 1) * H]
                    for h in range(nF):
                        nc.tensor.matmul(
                            out=sl, lhsT=aggT_sl(c, h, di),
                            rhs=C_(f"w1{d}{h}"),
                            start=(c == 0 and di == 0 and h == 0), stop=False,
                            skip_group_check=True)
                    nc.tensor.matmul(out=sl, lhsT=b1ones[di], rhs=b1r[di],
                                     start=False,
                                     stop=(c == NCHUNK - 1 and di == 1),
                                     skip_group_check=True)
            for c in range(NCHUNK):
                nc.vector.tensor_scalar(
                    out=cbt[c][:], in0=l1[:, c * 2 * H:(c + 1) * 2 * H],
                    scalar1=0.0, scalar2=None, op0=mx)

            # stage 4: o2T[d] [P(l1_d cols), G_cap] = cbt_d^T @ Pr; both
            # directions pack into one PSUM bank (single merged group, 16-col
            # stride) and leave PSUM in one copy.
            GS = 16
            o2 = ps2.tile([P, 2 * GS], dt, tag="ps2", name="o2ps")
            for di in range(2):
                for c in range(NCHUNK):
                    nc.tensor.matmul(out=o2[:, di * GS:di * GS + G_cap],
                                     lhsT=cbt[c][:, di * H:(di + 1) * H],
                                     rhs=C_(f"pr{c}"),
                                     start=(di == 0 and c == 0),
                                     stop=(di == 1 and c == NCHUNK - 1),
                                     skip_group_check=True)
            o2T = spool.tile([P, 2 * GS], dtb, tag="o2T", name="o2T")
            nc.vector.tensor_copy(out=o2T[:], in_=o2[:])

            # root-feature part: rfs[h] = relu(xrootT_h) * prsum (per column)
            rfs = []
            for h in range(nF):
                t = spool.tile([P, G_cap], dtb, tag=f"rfs{h}", name="rfs")
                nc.vector.scalar_tensor_tensor(
                    out=t[:], in0=C_(f"xrootT{h}"), scalar=0.0,
                    in1=C_("prsum"), op0=mx, op1=mul)
                rfs.append(t)

            # stage 5: tot[d] [H, G_cap] = relu(W2_d^T @ [rfs | o2T_d] + b2_d)
            # Both directions pack into one PSUM bank + one relu.
            tp = ps2.tile([P, 2 * GS], dt, tag="ps2", name="totps")
            for di, d in enumerate(("bu", "td")):
                sl = tp[:, di * GS:di * GS + G_cap]
                for h in range(nF):
                    nc.tensor.matmul(out=sl, lhsT=C_(f"w2rf{d}{h}"),
                                     rhs=rfs[h][:],
                                     start=(di == 0 and h == 0), stop=False,
                                     skip_group_check=True)
                nc.tensor.matmul(out=sl, lhsT=C_(f"w2l1{d}"),
                                 rhs=o2T[:, di * GS:di * GS + G_cap],
                                 start=False, stop=False,
                                 skip_group_check=True)
                nc.tensor.matmul(out=sl, lhsT=b2r[di],
                                 rhs=b2ones[di], start=False, stop=(di == 1),
                                 skip_group_check=True)
            tot = spool.tile([P, 2 * GS], dtb, tag="tot", name="tot")
            nc.vector.tensor_scalar(out=tot[:], in0=tp[:], scalar1=0.0,
                                    scalar2=None, op0=mx)

            # stage 6: logits [G_cap, C] = tot^T @ W_lin + b_lin
            lg = ps2.tile([G_cap, C], dt, tag="ps2", name="lgps")
            for di, d in enumerate(("bu", "td")):
                nc.tensor.matmul(out=lg[:], lhsT=tot[:, di * GS:di * GS + G_cap],
                                 rhs=C_(f"wl{d}"), start=(di == 0), stop=False)
            nc.tensor.matmul(out=lg[:], lhsT=blones,
                             rhs=blr, start=False, stop=True)

            # log_softmax rows: exp+accum on ACT, then ln(s) via the DVE
            # Horner polynomial above (logits are O(1): no max-subtraction).
            ez = spool.tile([G_cap, C], dt, tag="ez", name="ez")
            se = spool.tile([G_cap, 1], dt, tag="se", name="se")
            nc.scalar.activation(ez[:], lg[:], Exp, accum_out=se[:])
            uu_t = spool.tile([G_cap, 1], dt, tag="uu", name="uu")
            nc.vector.tensor_scalar(out=uu_t[:], in0=se[:], scalar1=0.25,
                                    scalar2=-1.0, op0=mul, op1=add)
            lse = spool.tile([G_cap, 1], dt, tag="lse", name="lse")
            nc.vector.tensor_scalar(out=lse[:], in0=uu_t[:],
                                    scalar1=float(lncoef[0]),
                                    scalar2=float(lncoef[1]), op0=mul, op1=add)
            for ck in lncoef[2:]:
                nc.vector.tensor_scalar(out=lse[:], in0=lse[:],
                                        scalar1=uu_t[:], scalar2=float(ck),
                                        op0=mul, op1=add)
            res = spool.tile([G_cap, C], dt, tag="res", name="res")
            nc.vector.tensor_scalar(out=res[:], in0=lg[:], scalar1=lse[:],
                                    scalar2=None, op0=sub)
            nc.sync.dma_start(out[:], res[:])

    nc.compile()
    return nc


_PROG_CACHE = {}


def _prepare_maps(inputs):
    in_maps, meta = _preprocess(inputs["x"], inputs["edge_index"],
                                inputs["batch"], inputs["num_graphs"])
    _pack_consts(in_maps, inputs, meta)
    meta["key"] = (meta["F"], meta["H"], meta["C"], meta["G_cap"], meta["Kc"])
    return in_maps, meta


def _prepare(inputs):
    in_maps, meta = _prepare_maps(inputs)
    key = meta["key"]
    if key not in _PROG_CACHE:
        _PROG_CACHE[key] = _build_program(*key)
    return _PROG_CACHE[key], in_maps, meta


def kernel(**inputs):
    from concourse.bass_utils import run_bass_kernel_spmd

    nc, in_maps, meta = _prepare(inputs)
    res = run_bass_kernel_spmd(nc, in_maps, list(range(NCORES)))
    G = meta["G"]
    cog, gl = meta["core_of_graph"], meta["glocal"]
    out = np.empty((G, meta["C"]), np.float32)
    for g in range(G):
        out[g] = res.results[cog[g]]["out"][gl[g]]
    return out
